# revision 1
# baseline (speedup 1.0000x reference)
"""Trainium2 Bass kernel for nn_ConvTransduce1D (self-contained).

Computes, for x [16, 4096, 128] fp32, the CTC-style automaton forward scores
out [16, 4096, 52] of 52 tiny lexicon automata (26 single-token [c], 26
two-token [c, c+1], c = 1..26, blank = 0) over sliding windows of K=5 frames
(stride 1, pad 2).

Closed form (validated against the jax reference):
  For window w, with padded frames e_t = xp[w+t] (t = 0..4):
    d^u_t = e_t[c] - e_t[0];  d^v_t = e_t[c+1] - e_t[0]
    Du = exp(d^u), Dv = exp(d^v), Sb = sum_t e_t[0]
  Linear-space recurrence over t (per window, per lexicon column):
    H += Ru;  Ru = (Ru+1)*Du_t;  Rv = (Rv+H)*Dv_t;  G2 += Rv
  out[:, 0:26] = ln(H + Ru) + Sb;  out[:, 26:52] = ln(G2) + Sb
fp32/bf16 linear space is safe: |path scores| <= ~30.

Sharding: pure data parallel — batch 16 split as 2 per core across 8 cores.
Host prep per shard: zero-pad time dim by 2 and slice channels 0..27 (the
only channels the automata read) -> x28p [2, 4100, 28] contiguous.

Perf: recurrence planes in bf16 (DVE 2x tensor_tensor / 4x tensor_scalar);
(Ru+1)*Du is tensor_scalar(+1)+tensor_tensor (scalar_tensor_tensor is
1x-only). XDEU/XDEV exp tiles are 28-col padded so t-shifted window reads
stay 4B-aligned. Pool engine carries the H prefix chain; ACT does exp/ln
and small copies. Plane tiles rotate (bufs=4) to avoid WAR serialization.
"""

from contextlib import ExitStack

import numpy as np

import concourse.bacc as bacc
import concourse.bass as bass
import concourse.mybir as mybir
import concourse.tile as tile
from concourse.bass_utils import run_bass_kernel_spmd

F32 = mybir.dt.float32
BF16 = mybir.dt.bfloat16
A = mybir.AluOpType
AF = mybir.ActivationFunctionType

B_FULL, T, C = 16, 4096, 128
KTAPS = 5
PAD = 2
TP = T + 2 * PAD
CH = 28          # channels shipped: blank + labels 1..27
NK = 26          # lexicon entries per type
NCOL = 52        # output channels
N_CORES = 8
B_CORE = B_FULL // N_CORES


def _mkap(ap, dims, extra_offset=0):
    """Manual AP on the same tensor: keep partition dim, replace free dims."""
    part = ap.ap[0]
    return bass.AP(ap.tensor, ap.offset + extra_offset,
                   [list(part)] + [list(d) for d in dims])


def _build_core_kernel(nc, w_pp=32, b_core=B_CORE, dt_rec=BF16):
    x = nc.declare_dram_parameter("x", [b_core, TP, CH], F32, isOutput=False)
    y = nc.declare_dram_parameter("y", [b_core, T, NCOL], F32, isOutput=True)

    n_chunks = T // (128 * w_pp)
    rows = w_pp + KTAPS - 1

    with ExitStack() as ctx:
        tc = ctx.enter_context(tile.TileContext(nc))
        pool = ctx.enter_context(tc.tile_pool(name="main", bufs=2))
        rot = ctx.enter_context(tc.tile_pool(name="rot", bufs=4))

        v = nc.vector
        g = nc.gpsimd
        s = nc.scalar

        for b in range(b_core):
            for c in range(n_chunks):
                base = c * 128 * w_pp
                X3 = pool.tile([128, rows, CH], F32, tag="X3")
                nc.sync.dma_start(
                    out=X3[:],
                    in_=bass.AP(x, (b * TP + base) * CH,
                                [[w_pp * CH, 128], [CH, rows], [1, CH]]))

                XD = pool.tile([128, rows, CH - 1], F32, tag="XD")
                v.tensor_tensor(XD[:], X3[:, :, 1:CH],
                                X3[:, :, 0:1].broadcast_to(
                                    [128, rows, CH - 1]), A.subtract)
                # aligned bf16 exp tiles (28-wide rows; cols 0:26 used)
                XU = pool.tile([128, rows, CH], dt_rec, tag="XU")
                XV = pool.tile([128, rows, CH], dt_rec, tag="XV")
                s.activation(XU[:, :, 0:NK], XD[:, :, 0:NK], AF.Exp)
                s.activation(XV[:, :, 0:NK], XD[:, :, 1:NK + 1], AF.Exp)

                Sb = pool.tile([128, w_pp], F32, tag="Sb")
                v.tensor_reduce(
                    Sb[:], _mkap(X3[:], [[CH, w_pp], [CH, KTAPS]]),
                    mybir.AxisListType.X, A.add)

                def Du(t):
                    return XU[:, t:t + w_pp, 0:NK]

                def Dv(t):
                    return XV[:, t:t + w_pp, 0:NK]

                def pt(tag):
                    return rot.tile([128, w_pp, NK], dt_rec, tag=tag,
                                    name=f"{tag}_t")

                # t = 0
                Ru = pt("Ru")
                v.tensor_copy(Ru[:], Du(0))
                # t = 1
                H = pt("H")
                v.tensor_copy(H[:], Ru[:])
                Rp = pt("Rp")
                v.tensor_scalar_add(Rp[:], Ru[:], 1.0)
                Ru = pt("Ru")
                v.tensor_tensor(Ru[:], Rp[:], Du(1), A.mult)
                Rv = pt("Rv")
                v.tensor_tensor(Rv[:], H[:], Dv(1), A.mult)
                G2 = pool.tile([128, w_pp, NK], dt_rec, tag="G2")
                s.activation(G2[:], Rv[:], AF.Copy)
                # t = 2..4
                for t in range(2, KTAPS):
                    Hn = pt("H")
                    g.tensor_tensor(Hn[:], H[:], Ru[:], A.add)
                    H = Hn
                    Rp = pt("Rp")
                    v.tensor_scalar_add(Rp[:], Ru[:], 1.0)
                    Run = pt("Ru")
                    v.tensor_tensor(Run[:], Rp[:], Du(t), A.mult)
                    Tt = pt("Tt")
                    v.tensor_tensor(Tt[:], Rv[:], H[:], A.add)
                    Rvn = pt("Rv")
                    v.tensor_tensor(Rvn[:], Tt[:], Dv(t), A.mult)
                    Ru, Rv = Run, Rvn
                    if t in (2, 3):
                        g.tensor_tensor(G2[:], G2[:], Rv[:], A.add)
                    else:
                        v.tensor_tensor(G2[:], G2[:], Rv[:], A.add)

                G1 = pt("Tt")
                v.tensor_tensor(G1[:], H[:], Ru[:], A.add)

                OUT = pool.tile([128, w_pp, NCOL], F32, tag="OUT")
                s.activation(OUT[:, :, 0:NK], G1[:], AF.Ln)
                s.activation(OUT[:, :, NK:NCOL], G2[:], AF.Ln)
                # Sb add split by type half so the type-1 half (and its
                # DMA) proceeds while Ln(G2) is still running
                sb_ap = _mkap(Sb[:], [[1, w_pp], [0, NK]])
                g.tensor_tensor(OUT[:, :, 0:NK], OUT[:, :, 0:NK], sb_ap, A.add)
                v.tensor_tensor(OUT[:, :, NK:NCOL], OUT[:, :, NK:NCOL],
                                sb_ap, A.add)

                nc.sync.dma_start(
                    out=bass.AP(y, b * T * NCOL + base * NCOL,
                                [[w_pp * NCOL, 128], [NCOL, w_pp], [1, NCOL]]),
                    in_=OUT[:])
    return nc


_NC_CACHE = {}


def _get_nc():
    if "nc" not in _NC_CACHE:
        nc = bacc.Bacc()
        _build_core_kernel(nc)
        nc.compile()
        _NC_CACHE["nc"] = nc
    return _NC_CACHE["nc"]


def _prep_shard(x_shard):
    """[B_CORE, T, C] -> zero-padded, channel-sliced [B_CORE, TP, CH]."""
    out = np.zeros((x_shard.shape[0], TP, CH), np.float32)
    out[:, PAD:PAD + T, :] = x_shard[:, :, 0:CH]
    return out


def _run(x, trace=False, **kw):
    x = np.asarray(x, dtype=np.float32)
    assert x.shape == (B_FULL, T, C), x.shape
    nc = _get_nc()
    in_maps = [{"x": _prep_shard(x[i * B_CORE:(i + 1) * B_CORE])}
               for i in range(N_CORES)]
    res = run_bass_kernel_spmd(nc, in_maps, list(range(N_CORES)),
                               trace=trace, **kw)
    out = np.concatenate([res.results[i]["y"] for i in range(N_CORES)], axis=0)
    return np.ascontiguousarray(out.astype(np.float32)), res


def kernel(x):
    out, _ = _run(x, trace=False)
    return out



# revision 4
# speedup vs baseline: 1.2337x; 1.2337x over previous
"""Trainium2 Bass kernel for nn_ConvTransduce1D (self-contained).

Computes, for x [16, 4096, 128] fp32, the CTC-style automaton forward scores
out [16, 4096, 52] of 52 tiny lexicon automata (26 single-token [c], 26
two-token [c, c+1], c = 1..26, blank = 0) over sliding windows of K=5 frames
(stride 1, pad 2).

Closed form (validated against the jax reference):
  For window w, padded frames give u_t = exp(x[w+t, c] - x[w+t, 0]) and
  v_t = exp(x[w+t, c+1] - x[w+t, 0]), t = 0..4; Sb = sum_t x[w+t, 0].
  e_t  = u_t * (1 + e_{t-1})        (runs of c ending at t;   e_0 = u_0)
  C_t  = C_{t-1} + e_t              (prefix sums;             C_0 = e_0)
  Rv_t = (Rv_{t-1} + C_{t-1}) * v_t (u-run then v-run to t;   Rv_1 = C_0*v_1)
  out[:, 0:26]  = ln(C_3 + e_4) + Sb
  out[:, 26:52] = ln(Rv_1 + Rv_2 + Rv_3 + Rv_4) + Sb
Linear-space bf16 is safe: |log path scores| <= ~30.

Sharding: pure data parallel, batch 16 -> 2 per core across 8 cores.
Host prep: slice channels 0..27 (all the automata read), zero-pad time by
2, cast to bf16 -> [2, 4100, 28] per core.  Output bf16 -> f32 on host.

Engine split (cost model: ~ns = free_size * cycle_t / speedup):
  DVE  : chain muls/adds (bf16 2x tt, 4x ts)
  Pool : XD subtract + Sb window-reduce + Sb broadcast-adds + overflow
         chain steps as fused scalar_tensor_tensor (0.6 eff)
  ACT  : one shared exp per batch (u/v are overlapping column views),
         Ln of G1 (SBUF) and G2 (PSUM)
  PE   : G2 = Rv_1+..+Rv_4 as identity-matmul accumulation into PSUM
"""

from contextlib import ExitStack

import numpy as np

import concourse.bacc as bacc
import concourse.bass as bass
import concourse.mybir as mybir
import concourse.tile as tile
from concourse.bass_utils import run_bass_kernel_spmd

F32 = mybir.dt.float32
BF16 = mybir.dt.bfloat16
A = mybir.AluOpType
AF = mybir.ActivationFunctionType

B_FULL, T, C = 16, 4096, 128
KTAPS = 5
PAD = 2
TP = T + 2 * PAD
CH = 28          # channels shipped: blank + labels 1..27
NK = 26          # lexicon entries per type
NCOL = 52        # output channels
N_CORES = 8
B_CORE = B_FULL // N_CORES
WPP = 32         # windows per partition (128 * 32 = 4096)
ROWS = WPP + KTAPS - 1


def _mkap(ap, dims, extra_offset=0):
    """Manual AP on the same tensor: keep partition dim, replace free dims."""
    part = ap.ap[0]
    return bass.AP(ap.tensor, ap.offset + extra_offset,
                   [list(part)] + [list(d) for d in dims])


# Engine assignment knobs (tuned against the TimelineSim cost model).
CFG = dict(
    use_pe=True,        # G2 sum via PE identity matmuls into PSUM
    e_step_eng=("v", "v", "v", "p"),   # e_1..e_4: v=DVE ts+tt, p=Pool stt
    a_add_eng=("v", "v", "p"),         # A_2..A_4 adds
    sb_add_eng="p",     # p=Pool stt, v=DVE 1x tt
    xd_eng="p",
)


def _build_core_kernel(nc, cfg=CFG, b_core=B_CORE):
    x = nc.declare_dram_parameter("x", [b_core, TP, CH], BF16, isOutput=False)
    ident = nc.declare_dram_parameter("ident", [128, 128], BF16, isOutput=False)
    y = nc.declare_dram_parameter("y", [b_core, T, NCOL], BF16, isOutput=True)

    with ExitStack() as ctx:
        tc = ctx.enter_context(tile.TileContext(nc))
        const = ctx.enter_context(tc.tile_pool(name="const", bufs=1))
        pool = ctx.enter_context(tc.tile_pool(name="main", bufs=2))
        rot = ctx.enter_context(tc.tile_pool(name="rot", bufs=4))
        psum = ctx.enter_context(tc.tile_pool(name="ps", bufs=2, space="PSUM"))

        v = nc.vector
        g = nc.gpsimd
        s = nc.scalar
        pe = nc.tensor

        if cfg["use_pe"]:
            ID = const.tile([128, 128], BF16, tag="ID")
            nc.sync.dma_start(out=ID[:], in_=ident.ap())

        # ---- stage 1: input DMA for both batches ----
        X3s = []
        for b in range(b_core):
            X3 = pool.tile([128, ROWS, CH], BF16, tag="X3", name=f"X3_{b}")
            nc.sync.dma_start(
                out=X3[:],
                in_=bass.AP(x, b * TP * CH,
                            [[WPP * CH, 128], [CH, ROWS], [1, CH]]))
            X3s.append(X3)

        # ---- stage 2: XD + Sb (Pool), exp (ACT) ----
        XDs, Sbs, EXs = [], [], []
        for b in range(b_core):
            X3 = X3s[b]
            XD = pool.tile([128, ROWS, CH], BF16, tag="XD", name=f"XD_{b}")
            if cfg["xd_eng"] == "p":
                g.scalar_tensor_tensor(
                    XD[:, :, 0:CH - 1], X3[:, :, 1:CH], 0.0,
                    X3[:, :, 0:1].broadcast_to([128, ROWS, CH - 1]),
                    A.bypass, A.subtract)
            else:
                v.tensor_tensor(XD[:, :, 0:CH - 1], X3[:, :, 1:CH],
                                X3[:, :, 0:1].broadcast_to(
                                    [128, ROWS, CH - 1]), A.subtract)
            XDs.append(XD)
            Sb = pool.tile([128, WPP], F32, tag="Sb", name=f"Sb_{b}")
            v.tensor_reduce(
                Sb[:], _mkap(X3[:], [[CH, WPP], [CH, KTAPS]]),
                mybir.AxisListType.X, A.add)
            Sbs.append(Sb)
        for b in range(b_core):
            EX = pool.tile([128, ROWS, CH], BF16, tag="EX", name=f"EX_{b}")
            s.activation(EX[:, :, 0:CH - 1], XDs[b][:, :, 0:CH - 1], AF.Exp)
            EXs.append(EX)

        # window views into EX: u_t cols 0:26 (labels 1..26), v_t cols 1:27
        def U(b, t):
            return EXs[b][:, t:t + WPP, 0:NK]

        def V(b, t):
            return EXs[b][:, t:t + WPP, 1:NK + 1]

        def pt(tag, b):
            return rot.tile([128, WPP, NK], BF16, tag=tag, name=f"{tag}_{b}")

        # ---- stage 3: chains, interleaved across batches ----
        e = [[None] * KTAPS for _ in range(b_core)]   # e_1..e_4 tiles
        Cp = [[None] * KTAPS for _ in range(b_core)]  # C_1..C_3 tiles
        Rv = [[None] * KTAPS for _ in range(b_core)]
        G1 = [None] * b_core
        G2ps = [None] * b_core

        # E-chain: e_j = (e_{j-1} + 1) * u_j
        for j in range(1, KTAPS):
            for b in range(b_core):
                prev = U(b, 0) if j == 1 else e[b][j - 1][:]
                eng = cfg["e_step_eng"][j - 1]
                ej = pt("e", b)
                if eng == "p":
                    g.scalar_tensor_tensor(ej[:], prev, 1.0, U(b, j),
                                           A.add, A.mult)
                else:
                    tmp = pt("t1", b)
                    v.tensor_scalar_add(tmp[:], prev, 1.0)
                    v.tensor_tensor(ej[:], tmp[:], U(b, j), A.mult)
                e[b][j] = ej

        # C-prefix + Rv chain + G2 accumulation
        for b in range(b_core):
            Rv1 = pt("Rv", b)
            v.tensor_tensor(Rv1[:], U(b, 0), V(b, 1), A.mult)
            Rv[b][1] = Rv1
        for j in range(1, 4):  # C_1..C_3
            for b in range(b_core):
                prev = U(b, 0) if j == 1 else Cp[b][j - 1][:]
                Cj = pt("C", b)
                v.tensor_tensor(Cj[:], prev, e[b][j][:], A.add)
                Cp[b][j] = Cj
        for k in range(2, KTAPS):  # Rv_2..Rv_4
            for b in range(b_core):
                Ak = pt("Ak", b)
                eng = cfg["a_add_eng"][k - 2]
                if eng == "p":
                    g.scalar_tensor_tensor(Ak[:], Rv[b][k - 1][:], 0.0,
                                           Cp[b][k - 1][:], A.bypass, A.add)
                else:
                    v.tensor_tensor(Ak[:], Rv[b][k - 1][:], Cp[b][k - 1][:],
                                    A.add)
                Rvk = pt("Rv", b)
                v.tensor_tensor(Rvk[:], Ak[:], V(b, k), A.mult)
                Rv[b][k] = Rvk

        for b in range(b_core):
            G1b = pt("G1", b)
            v.tensor_tensor(G1b[:], Cp[b][3][:], e[b][4][:], A.add)
            G1[b] = G1b

        # G2 = Rv_1 + Rv_2 + Rv_3 + Rv_4
        if cfg["use_pe"]:
            HNK = NK // 2
            for b in range(b_core):
                ps = [psum.tile([128, WPP, HNK], F32, tag=f"g2p{h}",
                                name=f"g2p{h}_{b}") for h in range(2)]
                for h in range(2):
                    c0 = h * HNK
                    for k in range(1, KTAPS):
                        pe.matmul(ps[h][:],
                                  lhsT=ID[:],
                                  rhs=Rv[b][k][:, :, c0:c0 + HNK],
                                  start=(k == 1), stop=(k == KTAPS - 1))
                G2ps[b] = ps
        else:
            for b in range(b_core):
                s1 = pt("s1", b)
                v.tensor_tensor(s1[:], Rv[b][1][:], Rv[b][2][:], A.add)
                s2 = pt("s2", b)
                v.tensor_tensor(s2[:], Rv[b][3][:], Rv[b][4][:], A.add)
                g2 = pt("G2", b)
                v.tensor_tensor(g2[:], s1[:], s2[:], A.add)
                G2ps[b] = g2

        # ---- stage 4: Ln, +Sb, output DMA ----
        for b in range(b_core):
            OUT = pool.tile([128, WPP, NCOL], BF16, tag="OUT", name=f"O_{b}")
            s.activation(OUT[:, :, 0:NK], G1[b][:], AF.Ln)
            if cfg["use_pe"]:
                HNK = NK // 2
                for h in range(2):
                    s.activation(OUT[:, :, NK + h * HNK:NK + (h + 1) * HNK],
                                 G2ps[b][h][:], AF.Ln)
            else:
                s.activation(OUT[:, :, NK:NCOL], G2ps[b][:], AF.Ln)

            sb_ap = _mkap(Sbs[b][:], [[1, WPP], [0, NK]])
            if cfg["sb_add_eng"] == "p":
                g.scalar_tensor_tensor(OUT[:, :, 0:NK], OUT[:, :, 0:NK], 0.0,
                                       sb_ap, A.bypass, A.add)
                g.scalar_tensor_tensor(OUT[:, :, NK:NCOL], OUT[:, :, NK:NCOL],
                                       0.0, sb_ap, A.bypass, A.add)
            else:
                v.tensor_tensor(OUT[:, :, 0:NK], OUT[:, :, 0:NK], sb_ap, A.add)
                v.tensor_tensor(OUT[:, :, NK:NCOL], OUT[:, :, NK:NCOL],
                                sb_ap, A.add)

            nc.sync.dma_start(
                out=bass.AP(y, b * T * NCOL,
                            [[WPP * NCOL, 128], [NCOL, WPP], [1, NCOL]]),
                in_=OUT[:])
    return nc


_NC_CACHE = {}


def _get_nc():
    if "nc" not in _NC_CACHE:
        nc = bacc.Bacc()
        _build_core_kernel(nc)
        nc.compile()
        _NC_CACHE["nc"] = nc
    return _NC_CACHE["nc"]


_BF16_NP = mybir.dt.np(BF16)


def _prep_shard(x_shard):
    """[B_CORE, T, C] f32 -> zero-padded, channel-sliced bf16 [B_CORE, TP, CH]."""
    out = np.zeros((x_shard.shape[0], TP, CH), _BF16_NP)
    out[:, PAD:PAD + T, :] = x_shard[:, :, 0:CH].astype(_BF16_NP)
    return out


def _run(x, trace=False, **kw):
    x = np.asarray(x, dtype=np.float32)
    assert x.shape == (B_FULL, T, C), x.shape
    nc = _get_nc()
    ident = np.eye(128, dtype=_BF16_NP)
    in_maps = [{"x": _prep_shard(x[i * B_CORE:(i + 1) * B_CORE]),
                "ident": ident}
               for i in range(N_CORES)]
    res = run_bass_kernel_spmd(nc, in_maps, list(range(N_CORES)),
                               trace=trace, **kw)
    out = np.concatenate([res.results[i]["y"] for i in range(N_CORES)], axis=0)
    return np.ascontiguousarray(out.astype(np.float32)), res


def kernel(x):
    out, _ = _run(x, trace=False)
    return out


# revision 7
# speedup vs baseline: 1.2821x; 1.0393x over previous
"""Trainium2 Bass kernel for nn_ConvTransduce1D (self-contained).

Computes, for x [16, 4096, 128] fp32, the CTC-style automaton forward scores
out [16, 4096, 52] of 52 tiny lexicon automata (26 single-token [c], 26
two-token [c, c+1], c = 1..26, blank = 0) over sliding windows of K=5 frames
(stride 1, pad 2).

Closed form (validated against the jax reference):
  For window w, padded frames give u_t = exp(x[w+t, c] - x[w+t, 0]) and
  v_t = exp(x[w+t, c+1] - x[w+t, 0]), t = 0..4; Sb = sum_t x[w+t, 0].
  e_t  = u_t * (1 + e_{t-1})        (runs of c ending at t;   e_0 = u_0)
  C_t  = C_{t-1} + e_t              (prefix sums;             C_0 = e_0)
  Rv_t = (Rv_{t-1} + C_{t-1}) * v_t (u-run then v-run to t;   Rv_1 = C_0*v_1)
  out[:, 0:26]  = ln(C_3 + e_4) + Sb
  out[:, 26:52] = ln(Rv_1 + Rv_2 + Rv_3 + Rv_4) + Sb
Linear-space bf16 is safe: |log path scores| <= ~30.

Sharding: pure data parallel, batch 16 -> 2 per core across 8 cores.
Host prep: slice channels 0..27 (all the automata read), zero-pad time by
2, cast to bf16 -> [2, 4100, 28] per core.  Output bf16 -> f32 on host.

Engine split (cost model: ~ns = free_size * cycle_t / speedup):
  DVE  : chain muls/adds (bf16 2x tt, 4x ts)
  Pool : XD subtract + Sb window-reduce + Sb broadcast-adds + overflow
         chain steps as fused scalar_tensor_tensor (0.6 eff)
  ACT  : one shared exp per batch (u/v are overlapping column views),
         Ln of G1 (SBUF) and G2 (PSUM)
  PE   : G2 = Rv_1+..+Rv_4 as identity-matmul accumulation into PSUM
"""

from contextlib import ExitStack

import numpy as np

import concourse.bacc as bacc
import concourse.bass as bass
import concourse.mybir as mybir
import concourse.tile as tile
from concourse.bass_utils import run_bass_kernel_spmd

F32 = mybir.dt.float32
BF16 = mybir.dt.bfloat16
A = mybir.AluOpType
AF = mybir.ActivationFunctionType

B_FULL, T, C = 16, 4096, 128
KTAPS = 5
PAD = 2
TP = T + 2 * PAD
CH = 28          # channels shipped: blank + labels 1..27
NK = 26          # lexicon entries per type
NCOL = 52        # output channels
N_CORES = 8
B_CORE = B_FULL // N_CORES
WPP = 32         # windows per partition (128 * 32 = 4096)
ROWS = WPP + KTAPS - 1


def _mkap(ap, dims, extra_offset=0):
    """Manual AP on the same tensor: keep partition dim, replace free dims."""
    part = ap.ap[0]
    return bass.AP(ap.tensor, ap.offset + extra_offset,
                   [list(part)] + [list(d) for d in dims])


# Engine assignment knobs (tuned against the TimelineSim cost model).
CFG = dict(
    use_pe=True,        # G2 sum via PE identity matmuls into PSUM
    e_step_eng=("v", "v", "v", "p"),   # e_1..e_4: v=DVE ts+tt, p=Pool stt
    a_add_eng=("v", "v", "p"),         # A_2..A_4 adds
    sb_add_eng=("v", "p"),             # per-type engines for the +Sb adds
    xd_eng="p",
)


def _build_core_kernel(nc, cfg=CFG, b_core=B_CORE):
    x = nc.declare_dram_parameter("x", [b_core, TP, CH], BF16, isOutput=False)
    ident = nc.declare_dram_parameter("ident", [128, 128], BF16, isOutput=False)
    y = nc.declare_dram_parameter("y", [b_core, T, NCOL], BF16, isOutput=True)

    with ExitStack() as ctx:
        tc = ctx.enter_context(tile.TileContext(nc))
        const = ctx.enter_context(tc.tile_pool(name="const", bufs=1))
        pool = ctx.enter_context(tc.tile_pool(name="main", bufs=2))
        rot = ctx.enter_context(tc.tile_pool(name="rot", bufs=4))
        psum = ctx.enter_context(tc.tile_pool(name="ps", bufs=2, space="PSUM"))

        v = nc.vector
        g = nc.gpsimd
        s = nc.scalar
        pe = nc.tensor

        # ---- stage 1: input DMA for both batches first, ident last ----
        X3s = []
        for b in range(b_core):
            X3 = pool.tile([128, ROWS, CH], BF16, tag="X3", name=f"X3_{b}")
            nc.sync.dma_start(
                out=X3[:],
                in_=bass.AP(x, b * TP * CH,
                            [[WPP * CH, 128], [CH, ROWS], [1, CH]]))
            X3s.append(X3)
        if cfg["use_pe"]:
            ID = const.tile([128, 128], BF16, tag="ID")
            nc.sync.dma_start(out=ID[:], in_=ident.ap())

        # ---- stage 2: XD + exp per batch, interleaved for earliest start ----
        XDs, Sbs, EXs = [], [], []
        for b in range(b_core):
            X3 = X3s[b]
            XD = pool.tile([128, ROWS, CH], BF16, tag="XD", name=f"XD_{b}")
            eng = g if cfg["xd_eng"] == "p" else v
            eng.scalar_tensor_tensor(
                XD[:, :, 0:CH - 1], X3[:, :, 1:CH], 0.0,
                X3[:, :, 0:1].broadcast_to([128, ROWS, CH - 1]),
                A.bypass, A.subtract)
            XDs.append(XD)
            EX = pool.tile([128, ROWS, CH], BF16, tag="EX", name=f"EX_{b}")
            s.activation(EX[:, :, 0:CH - 1], XD[:, :, 0:CH - 1], AF.Exp)
            EXs.append(EX)
        for b in range(b_core):
            Sb = pool.tile([128, WPP], F32, tag="Sb", name=f"Sb_{b}")
            v.tensor_reduce(
                Sb[:], _mkap(X3s[b][:], [[CH, WPP], [CH, KTAPS]]),
                mybir.AxisListType.X, A.add)
            Sbs.append(Sb)

        # window views into EX: u_t cols 0:26 (labels 1..26), v_t cols 1:27
        def U(b, t):
            return EXs[b][:, t:t + WPP, 0:NK]

        def V(b, t):
            return EXs[b][:, t:t + WPP, 1:NK + 1]

        def pt(tag, b):
            return rot.tile([128, WPP, NK], BF16, tag=tag, name=f"{tag}_{b}")

        # ---- stage 3: chains, batch-interleaved step by step ----
        e = [[None] * KTAPS for _ in range(b_core)]   # e_1..e_4 tiles
        Cp = [[None] * KTAPS for _ in range(b_core)]  # C_1..C_3 tiles
        Rv = [[None] * KTAPS for _ in range(b_core)]
        G1 = [None] * b_core
        G2ps = [None] * b_core
        HNK = NK // 2
        if cfg["use_pe"]:
            for b in range(b_core):
                G2ps[b] = [psum.tile([128, WPP, HNK], F32, tag=f"g2p{h}",
                                     name=f"g2p{h}_{b}") for h in range(2)]

        def e_step(b, j):
            prev = U(b, 0) if j == 1 else e[b][j - 1][:]
            ej = pt("e", b)
            if cfg["e_step_eng"][j - 1] == "p":
                g.scalar_tensor_tensor(ej[:], prev, 1.0, U(b, j),
                                       A.add, A.mult)
            else:
                tmp = pt("t1", b)
                v.tensor_scalar_add(tmp[:], prev, 1.0)
                v.tensor_tensor(ej[:], tmp[:], U(b, j), A.mult)
            e[b][j] = ej

        def c_step(b, j):
            prev = U(b, 0) if j == 1 else Cp[b][j - 1][:]
            Cj = pt("C", b)
            v.tensor_tensor(Cj[:], prev, e[b][j][:], A.add)
            Cp[b][j] = Cj

        def rv_step(b, k):
            if k == 1:
                Rv1 = pt("Rv", b)
                v.tensor_tensor(Rv1[:], U(b, 0), V(b, 1), A.mult)
                Rv[b][1] = Rv1
                return
            Ak = pt("Ak", b)
            if cfg["a_add_eng"][k - 2] == "p":
                g.scalar_tensor_tensor(Ak[:], Rv[b][k - 1][:], 0.0,
                                       Cp[b][k - 1][:], A.bypass, A.add)
            else:
                v.tensor_tensor(Ak[:], Rv[b][k - 1][:], Cp[b][k - 1][:], A.add)
            Rvk = pt("Rv", b)
            v.tensor_tensor(Rvk[:], Ak[:], V(b, k), A.mult)
            Rv[b][k] = Rvk

        def pe_accum(b, k):
            if not cfg["use_pe"]:
                return
            for h in range(2):
                pe.matmul(G2ps[b][h][:], lhsT=ID[:],
                          rhs=Rv[b][k][:, :, h * HNK:(h + 1) * HNK],
                          start=(k == 1), stop=(k == KTAPS - 1))

        for b in range(b_core):
            e_step(b, 1)
        for b in range(b_core):
            rv_step(b, 1)
            pe_accum(b, 1)
        for b in range(b_core):
            c_step(b, 1)
            e_step(b, 2)
        for b in range(b_core):
            rv_step(b, 2)
            pe_accum(b, 2)
        for b in range(b_core):
            c_step(b, 2)
            e_step(b, 3)
        for b in range(b_core):
            rv_step(b, 3)
            pe_accum(b, 3)
        for b in range(b_core):
            c_step(b, 3)
            e_step(b, 4)
        for b in range(b_core):
            rv_step(b, 4)
            pe_accum(b, 4)
        for b in range(b_core):
            G1b = pt("G1", b)
            v.tensor_tensor(G1b[:], Cp[b][3][:], e[b][4][:], A.add)
            G1[b] = G1b
        if not cfg["use_pe"]:
            for b in range(b_core):
                s1 = pt("s1", b)
                v.tensor_tensor(s1[:], Rv[b][1][:], Rv[b][2][:], A.add)
                s2 = pt("s2", b)
                v.tensor_tensor(s2[:], Rv[b][3][:], Rv[b][4][:], A.add)
                g2 = pt("G2", b)
                v.tensor_tensor(g2[:], s1[:], s2[:], A.add)
                G2ps[b] = g2

        # ---- stage 4: Ln, +Sb, output DMA (per batch, earliest first) ----
        for b in range(b_core):
            OUT = pool.tile([128, WPP, NCOL], BF16, tag="OUT", name=f"O_{b}")
            s.activation(OUT[:, :, 0:NK], G1[b][:], AF.Ln)
            if cfg["use_pe"]:
                for h in range(2):
                    s.activation(OUT[:, :, NK + h * HNK:NK + (h + 1) * HNK],
                                 G2ps[b][h][:], AF.Ln)
            else:
                s.activation(OUT[:, :, NK:NCOL], G2ps[b][:], AF.Ln)

            sb_ap = _mkap(Sbs[b][:], [[1, WPP], [0, NK]])
            for half, eng_key in enumerate(cfg["sb_add_eng"]):
                sl = OUT[:, :, 0:NK] if half == 0 else OUT[:, :, NK:NCOL]
                if eng_key == "p":
                    g.scalar_tensor_tensor(sl, sl, 0.0, sb_ap,
                                           A.bypass, A.add)
                else:
                    v.tensor_tensor(sl, sl, sb_ap, A.add)

            nc.sync.dma_start(
                out=bass.AP(y, b * T * NCOL,
                            [[WPP * NCOL, 128], [NCOL, WPP], [1, NCOL]]),
                in_=OUT[:])
    return nc


_NC_CACHE = {}


def _get_nc():
    if "nc" not in _NC_CACHE:
        nc = bacc.Bacc()
        _build_core_kernel(nc)
        nc.compile()
        _NC_CACHE["nc"] = nc
    return _NC_CACHE["nc"]


_BF16_NP = mybir.dt.np(BF16)


def _prep_shard(x_shard):
    """[B_CORE, T, C] f32 -> zero-padded, channel-sliced bf16 [B_CORE, TP, CH]."""
    out = np.zeros((x_shard.shape[0], TP, CH), _BF16_NP)
    out[:, PAD:PAD + T, :] = x_shard[:, :, 0:CH].astype(_BF16_NP)
    return out


def _run(x, trace=False, **kw):
    x = np.asarray(x, dtype=np.float32)
    assert x.shape == (B_FULL, T, C), x.shape
    nc = _get_nc()
    ident = np.eye(128, dtype=_BF16_NP)
    in_maps = [{"x": _prep_shard(x[i * B_CORE:(i + 1) * B_CORE]),
                "ident": ident}
               for i in range(N_CORES)]
    res = run_bass_kernel_spmd(nc, in_maps, list(range(N_CORES)),
                               trace=trace, **kw)
    out = np.concatenate([res.results[i]["y"] for i in range(N_CORES)], axis=0)
    return np.ascontiguousarray(out.astype(np.float32)), res


def kernel(x):
    out, _ = _run(x, trace=False)
    return out


# revision 10
# speedup vs baseline: 1.3010x; 1.0147x over previous
"""Trainium2 Bass kernel for nn_ConvTransduce1D (self-contained).

Computes, for x [16, 4096, 128] fp32, the CTC-style automaton forward scores
out [16, 4096, 52] of 52 tiny lexicon automata (26 single-token [c], 26
two-token [c, c+1], c = 1..26, blank = 0) over sliding windows of K=5 frames
(stride 1, pad 2).

Closed form (validated against the jax reference):
  For window w, padded frames give u_t = exp(x[w+t, c] - x[w+t, 0]) and
  v_t = exp(x[w+t, c+1] - x[w+t, 0]), t = 0..4; Sb = sum_t x[w+t, 0].
  e_t  = u_t * (1 + e_{t-1})        (runs of c ending at t;   e_0 = u_0)
  C_t  = C_{t-1} + e_t              (prefix sums;             C_0 = e_0)
  Rv_t = (Rv_{t-1} + C_{t-1}) * v_t (u-run then v-run to t;   Rv_1 = C_0*v_1)
  out[:, 0:26]  = ln(C_3 + e_4) + Sb
  out[:, 26:52] = ln(Rv_1 + Rv_2 + Rv_3 + Rv_4) + Sb
Linear-space bf16 is safe: |log path scores| <= ~30.

Sharding: pure data parallel, batch 16 -> 2 per core across 8 cores.
Host prep: slice channels 0..27 (all the automata read), zero-pad time by
2, cast to bf16 -> [2, 4100, 28] per core.  Output bf16 -> f32 on host.

Engine split (cost model: ~ns = free_size * cycle_t / speedup):
  DVE  : chain muls/adds (bf16 2x tt, 4x ts)
  Pool : XD subtract + Sb window-reduce + Sb broadcast-adds + overflow
         chain steps as fused scalar_tensor_tensor (0.6 eff)
  ACT  : one shared exp per batch (u/v are overlapping column views),
         Ln of G1 (SBUF) and G2 (PSUM)
  PE   : G2 = Rv_1+..+Rv_4 as identity-matmul accumulation into PSUM
"""

from contextlib import ExitStack

import numpy as np

import concourse.bacc as bacc
import concourse.bass as bass
import concourse.mybir as mybir
import concourse.tile as tile
from concourse.bass_utils import run_bass_kernel_spmd

F32 = mybir.dt.float32
BF16 = mybir.dt.bfloat16
A = mybir.AluOpType
AF = mybir.ActivationFunctionType

B_FULL, T, C = 16, 4096, 128
KTAPS = 5
PAD = 2
TP = T + 2 * PAD
CH = 28          # channels shipped: blank + labels 1..27
NK = 26          # lexicon entries per type
NCOL = 52        # output channels
N_CORES = 8
B_CORE = B_FULL // N_CORES
WPP = 32         # windows per partition (128 * 32 = 4096)
ROWS = WPP + KTAPS - 1


def _mkap(ap, dims, extra_offset=0):
    """Manual AP on the same tensor: keep partition dim, replace free dims."""
    part = ap.ap[0]
    return bass.AP(ap.tensor, ap.offset + extra_offset,
                   [list(part)] + [list(d) for d in dims])


# Engine assignment knobs (tuned against the TimelineSim cost model).
CFG = dict(
    use_pe=True,        # G2 sum via PE identity matmuls into PSUM
    e_step_eng=("v", "v", "v", "p"),   # e_1..e_4: v=DVE ts+tt, p=Pool stt
    a_add_eng=("v", "v", "p"),         # A_2..A_4 adds
    g1_scale_eng="v",   # G1 *= exp(Sb): v=DVE 1x tt, p=Pool stt
    xd_eng=("v", "p"),  # per-batch: b0 on DVE (idle during fill), b1 Pool
)


def _build_core_kernel(nc, cfg=CFG, b_core=B_CORE):
    x = nc.declare_dram_parameter("x", [b_core, TP, CH], BF16, isOutput=False)
    ident = nc.declare_dram_parameter("ident", [128, 128], BF16, isOutput=False)
    y = nc.declare_dram_parameter("y", [b_core, T, NCOL], BF16, isOutput=True)

    with ExitStack() as ctx:
        tc = ctx.enter_context(tile.TileContext(nc))
        const = ctx.enter_context(tc.tile_pool(name="const", bufs=1))
        pool = ctx.enter_context(tc.tile_pool(name="main", bufs=2))
        rot = ctx.enter_context(tc.tile_pool(name="rot", bufs=4))
        psum = ctx.enter_context(tc.tile_pool(name="ps", bufs=2, space="PSUM"))

        v = nc.vector
        g = nc.gpsimd
        s = nc.scalar
        pe = nc.tensor

        # ---- stage 1: input DMA for both batches first, ident last ----
        X3s = []
        for b in range(b_core):
            X3 = pool.tile([128, ROWS, CH], BF16, tag="X3", name=f"X3_{b}")
            nc.sync.dma_start(
                out=X3[:],
                in_=bass.AP(x, b * TP * CH,
                            [[WPP * CH, 128], [CH, ROWS], [1, CH]]))
            X3s.append(X3)
        if cfg["use_pe"]:
            ID = const.tile([128, 128], BF16, tag="ID")
            nc.sync.dma_start(out=ID[:], in_=ident.ap())

        # ---- stage 2: XD + exp per batch, interleaved for earliest start ----
        XDs, ESbs, EXs = [], [], []
        for b in range(b_core):
            X3 = X3s[b]
            XD = pool.tile([128, ROWS, CH], BF16, tag="XD", name=f"XD_{b}")
            eng = g if cfg["xd_eng"][b] == "p" else v
            eng.scalar_tensor_tensor(
                XD[:, :, 0:CH - 1], X3[:, :, 1:CH], 0.0,
                X3[:, :, 0:1].broadcast_to([128, ROWS, CH - 1]),
                A.bypass, A.subtract)
            XDs.append(XD)
            EX = pool.tile([128, ROWS, CH], BF16, tag="EX", name=f"EX_{b}")
            s.activation(EX[:, :, 0:CH - 1], XD[:, :, 0:CH - 1], AF.Exp)
            EXs.append(EX)
        for b in range(b_core):
            Sb = pool.tile([128, WPP], F32, tag="Sb", name=f"Sb_{b}")
            v.tensor_reduce(
                Sb[:], _mkap(X3s[b][:], [[CH, WPP], [CH, KTAPS]]),
                mybir.AxisListType.X, A.add)
            ESb = pool.tile([128, WPP], F32, tag="ESb", name=f"ESb_{b}")
            s.activation(ESb[:], Sb[:], AF.Exp)
            ESbs.append(ESb)

        # window views into EX: u_t cols 0:26 (labels 1..26), v_t cols 1:27
        def U(b, t):
            return EXs[b][:, t:t + WPP, 0:NK]

        def V(b, t):
            return EXs[b][:, t:t + WPP, 1:NK + 1]

        def pt(tag, b):
            return rot.tile([128, WPP, NK], BF16, tag=tag, name=f"{tag}_{b}")

        # ---- stage 3: chains, batch-interleaved step by step ----
        e = [[None] * KTAPS for _ in range(b_core)]   # e_1..e_4 tiles
        Cp = [[None] * KTAPS for _ in range(b_core)]  # C_1..C_3 tiles
        Rv = [[None] * KTAPS for _ in range(b_core)]
        G1 = [None] * b_core
        G2ps = [None] * b_core
        HNK = NK // 2
        if cfg["use_pe"]:
            for b in range(b_core):
                G2ps[b] = [psum.tile([128, WPP, HNK], F32, tag=f"g2p{h}",
                                     name=f"g2p{h}_{b}") for h in range(2)]

        def e_step(b, j):
            prev = U(b, 0) if j == 1 else e[b][j - 1][:]
            ej = pt("e", b)
            if cfg["e_step_eng"][j - 1] == "p":
                g.scalar_tensor_tensor(ej[:], prev, 1.0, U(b, j),
                                       A.add, A.mult)
            else:
                tmp = pt("t1", b)
                v.tensor_scalar_add(tmp[:], prev, 1.0)
                v.tensor_tensor(ej[:], tmp[:], U(b, j), A.mult)
            e[b][j] = ej

        def c_step(b, j):
            prev = U(b, 0) if j == 1 else Cp[b][j - 1][:]
            Cj = pt("C", b)
            v.tensor_tensor(Cj[:], prev, e[b][j][:], A.add)
            Cp[b][j] = Cj

        def rv_step(b, k):
            if k == 1:
                Rv1 = pt("Rv", b)
                v.tensor_tensor(Rv1[:], U(b, 0), V(b, 1), A.mult)
                Rv[b][1] = Rv1
                return
            Ak = pt("Ak", b)
            if cfg["a_add_eng"][k - 2] == "p":
                g.scalar_tensor_tensor(Ak[:], Rv[b][k - 1][:], 0.0,
                                       Cp[b][k - 1][:], A.bypass, A.add)
            else:
                v.tensor_tensor(Ak[:], Rv[b][k - 1][:], Cp[b][k - 1][:], A.add)
            Rvk = pt("Rv", b)
            v.tensor_tensor(Rvk[:], Ak[:], V(b, k), A.mult)
            Rv[b][k] = Rvk

        def pe_accum(b, k):
            if not cfg["use_pe"]:
                return
            for h in range(2):
                pe.matmul(G2ps[b][h][:], lhsT=ID[:],
                          rhs=Rv[b][k][:, :, h * HNK:(h + 1) * HNK],
                          start=(k == 1), stop=(k == KTAPS - 1))

        for b in range(b_core):
            e_step(b, 1)
        for b in range(b_core):
            rv_step(b, 1)
            pe_accum(b, 1)
        for b in range(b_core):
            c_step(b, 1)
            e_step(b, 2)
        for b in range(b_core):
            rv_step(b, 2)
            pe_accum(b, 2)
        for b in range(b_core):
            c_step(b, 2)
            e_step(b, 3)
        for b in range(b_core):
            rv_step(b, 3)
            pe_accum(b, 3)
        for b in range(b_core):
            c_step(b, 3)
            e_step(b, 4)
        for b in range(b_core):
            rv_step(b, 4)
            pe_accum(b, 4)
        if not cfg["use_pe"]:
            for b in range(b_core):
                s1 = pt("s1", b)
                v.tensor_tensor(s1[:], Rv[b][1][:], Rv[b][2][:], A.add)
                s2 = pt("s2", b)
                v.tensor_tensor(s2[:], Rv[b][3][:], Rv[b][4][:], A.add)
                g2 = pt("G2", b)
                v.tensor_tensor(g2[:], s1[:], s2[:], A.add)
                G2ps[b] = [g2]

        # ---- stage 4: G1 = C3+e4, fold exp(Sb) in linear space, Ln, DMA ----
        for b in range(b_core):
            esb_ap = _mkap(ESbs[b][:], [[1, WPP], [0, NK]])
            esb_h = [_mkap(ESbs[b][:], [[1, WPP], [0, HNK]]) for _ in range(2)]
            G1b = pt("G1", b)
            v.tensor_tensor(G1b[:], Cp[b][3][:], e[b][4][:], A.add)
            G1s = pt("G1s", b)
            if cfg["g1_scale_eng"] == "p":
                g.scalar_tensor_tensor(G1s[:], G1b[:], 0.0, esb_ap,
                                       A.bypass, A.mult)
            else:
                v.tensor_tensor(G1s[:], G1b[:], esb_ap, A.mult)
            G2s = pt("G2s", b)
            if cfg["use_pe"]:
                for h in range(2):
                    g.scalar_tensor_tensor(
                        G2s[:, :, h * HNK:(h + 1) * HNK], G2ps[b][h][:], 0.0,
                        esb_h[h], A.bypass, A.mult)
            else:
                g.scalar_tensor_tensor(G2s[:], G2ps[b][0][:], 0.0, esb_ap,
                                       A.bypass, A.mult)
            G1[b] = G1s
            G2ps[b] = G2s

        for b in range(b_core):
            OUT = pool.tile([128, WPP, NCOL], BF16, tag="OUT", name=f"O_{b}")
            s.activation(OUT[:, :, 0:NK], G1[b][:], AF.Ln)
            s.activation(OUT[:, :, NK:NCOL], G2ps[b][:], AF.Ln)
            nc.sync.dma_start(
                out=bass.AP(y, b * T * NCOL,
                            [[WPP * NCOL, 128], [NCOL, WPP], [1, NCOL]]),
                in_=OUT[:])
    return nc


_NC_CACHE = {}


def _get_nc():
    if "nc" not in _NC_CACHE:
        nc = bacc.Bacc()
        _build_core_kernel(nc)
        nc.compile()
        _NC_CACHE["nc"] = nc
    return _NC_CACHE["nc"]


_BF16_NP = mybir.dt.np(BF16)


def _prep_shard(x_shard):
    """[B_CORE, T, C] f32 -> zero-padded, channel-sliced bf16 [B_CORE, TP, CH]."""
    out = np.zeros((x_shard.shape[0], TP, CH), _BF16_NP)
    out[:, PAD:PAD + T, :] = x_shard[:, :, 0:CH].astype(_BF16_NP)
    return out


def _run(x, trace=False, **kw):
    x = np.asarray(x, dtype=np.float32)
    assert x.shape == (B_FULL, T, C), x.shape
    nc = _get_nc()
    ident = np.eye(128, dtype=_BF16_NP)
    in_maps = [{"x": _prep_shard(x[i * B_CORE:(i + 1) * B_CORE]),
                "ident": ident}
               for i in range(N_CORES)]
    res = run_bass_kernel_spmd(nc, in_maps, list(range(N_CORES)),
                               trace=trace, **kw)
    out = np.concatenate([res.results[i]["y"] for i in range(N_CORES)], axis=0)
    return np.ascontiguousarray(out.astype(np.float32)), res


def kernel(x):
    out, _ = _run(x, trace=False)
    return out
